# revision 2
# baseline (speedup 1.0000x reference)
"""DbrxAttention (B=1, S=2048, D=6144, 48 q heads / 8 kv heads, rope, causal)
on 8 Trainium2 NeuronCores.

Sharding: tensor-parallel across heads. Core c owns q heads [6c, 6c+6) and kv
head c. Wqkv output dim and Wout input dim are sharded; ReduceScatter after
out_proj sums partials; the host concatenates the row-shards.

v2: software-pipelined emission. The kernel runs 6 "periods"; period p
interleaves three instruction streams on the tensor engine so it never
stalls on softmax/eviction latency and stays at max p-state:
  - S1(p):   qkv projection for chunk p (two 4-bank passes over the 48
             contraction tiles; pass A produces k, v, q0, q1; pass B q2..q5)
  - S2(p-1): attention for chunk p-1 (scores -> exp -> scale -> xbar
             transpose -> probs@V), 6 heads
  - S3(p-2): out-projection for chunk p-2 + per-half ReduceScatter
PSUM: 4 banks stage1 accumulators + 2 score banks + 2 shared pv/outproj
banks = 8. The hidden chunk is resident in SBUF (read by both passes);
weights stream. All bulk DMA on the gpsimd SWDGE path (spreads across the
16 queues), latency-critical xbar transposes on the sync HWDGE queue.
"""

import numpy as np

N_CORES = 8
S = 2048
D = 6144
HD = 128
NQH = 6                 # q heads per core
P = 128
NKT = S // P            # 16 key tiles
NQC = 4                 # q chunks (periods of the attention pipeline)
QCW = S // NQC          # 512
DT = D // P             # 48 d-model contraction tiles
NG = DT // 2            # 24 kt-pairs
SCALE = HD ** -0.5
CAP = 12.0              # softmax constant shift
CLIP = 8.0

MM_NS = 0.43            # ns per moving row at max p-state

_cached_nc = None


def _build_nc():
    import concourse.mybir as mybir
    import concourse.tile as tile
    from concourse import bacc

    f16, f32 = mybir.dt.float16, mybir.dt.float32
    add_op = mybir.AluOpType.add
    mult_op = mybir.AluOpType.mult
    min_op = mybir.AluOpType.min
    max_op = mybir.AluOpType.max
    X = mybir.AxisListType.X
    Exp = mybir.ActivationFunctionType.Exp

    nc = bacc.Bacc("TRN2", target_bir_lowering=False, debug=False,
                   num_devices=N_CORES)

    # host-packed layouts (see kernel() for the packing)
    hiddenR = nc.dram_tensor("hiddenR", [P, DT, S], f16,
                             kind="ExternalInput").ap()
    wqkvR = nc.dram_tensor("wqkvR", [P, NG, 2, 1024], f16,
                           kind="ExternalInput").ap()
    woutR = nc.dram_tensor("woutR", [P, NG, 2 * NQH * P], f16,
                           kind="ExternalInput").ap()
    ccq = nc.dram_tensor("ccq", [P, S], f16, kind="ExternalInput").ap()
    ssq = nc.dram_tensor("ssq", [P, S], f16, kind="ExternalInput").ap()
    cck = nc.dram_tensor("cck", [P, S], f16, kind="ExternalInput").ap()
    ssk = nc.dram_tensor("ssk", [P, S], f16, kind="ExternalInput").ap()
    ident = nc.dram_tensor("ident", [P, P], f16, kind="ExternalInput").ap()
    maskd = nc.dram_tensor("maskd", [P, P], f16, kind="ExternalInput").ap()
    outs = [nc.dram_tensor(f"out{g}", [D // N_CORES, QCW], f16,
                           kind="ExternalOutput").ap() for g in range(NQC)]

    with tile.TileContext(nc) as tc:
        with (
            tc.tile_pool(name="const", bufs=1) as const,
            tc.tile_pool(name="kv", bufs=1) as kvp,
            tc.tile_pool(name="stream", bufs=1) as stream,
            tc.tile_pool(name="work", bufs=1) as work,
            tc.tile_pool(name="stats", bufs=1) as stats,
            tc.tile_pool(name="ps", bufs=1, space="PSUM") as psp,
            tc.tile_pool(name="dram", bufs=1, space="DRAM") as dram,
        ):
            ident_sb = const.tile([P, P], f16, tag="ident")
            nc.sync.dma_start(ident_sb[:], ident[:])
            maskd_sb = const.tile([P, P], f16, tag="maskd")
            nc.sync.dma_start(maskd_sb[:], maskd[:])
            negcap = const.tile([P, 1], f32, tag="negcap")
            nc.vector.memset(negcap[:], -CAP)

            k_sb = kvp.tile([P, S], f16, tag="k_sb")
            v_sb = kvp.tile([P, NKT, P], f16, tag="v_sb")

            # ---------------- emission machinery ----------------
            def merge(*streams):
                streams = [list(s) for s in streams if s]
                tot = [max(1.0, sum(ns for ns, _ in s)) for s in streams]
                done = [0.0] * len(streams)
                idx = [0] * len(streams)
                while True:
                    best = None
                    bf = None
                    for i, s in enumerate(streams):
                        if idx[i] >= len(s):
                            continue
                        f = done[i] / tot[i]
                        if bf is None or f < bf:
                            best, bf = i, f
                    if best is None:
                        return
                    ns, fn = streams[best][idx[best]]
                    fn()
                    done[best] += ns
                    idx[best] += 1

            # ---------------- stage 1: qkv + rope ----------------
            W_LOOKAHEAD = 4

            def make_s1(qc):
                """Items for chunk qc's qkv projection + rope."""
                cs = slice(QCW * qc, QCW * (qc + 1))
                st = {}
                items = []

                def start_chunk():
                    tabs = {}
                    for nm, src in (("ccq", ccq), ("ssq", ssq),
                                    ("cck", cck), ("ssk", ssk)):
                        t = stream.tile([P, QCW], f16, tag=nm, bufs=2,
                                        name=nm)
                        nc.sync.dma_start(t[:], src[:, cs])
                        tabs[nm] = t
                    st["tabs"] = tabs
                    st["q"] = work.tile([P, NQH, QCW], f16, tag="q_qc",
                                        bufs=2, name="q_qc")
                    st["ps1"] = [psp.tile([P, QCW], f32, tag=f"s1_{m}",
                                          bufs=1, name=f"s1_{m}")
                                 for m in range(4)]
                items.append((0, start_chunk))

                def w_load(half, g):
                    def fn():
                        t = stream.tile([P, 2, QCW], f16, tag="wt", bufs=6,
                                        name="w_t")
                        nc.gpsimd.dma_start(
                            t[:], wqkvR[:, g, :, QCW * half:QCW * (half + 1)])
                        st[("w", half, g)] = t
                    return fn

                def kt_group(half, g):
                    def fn():
                        la = g + W_LOOKAHEAD
                        if la < NG:
                            w_load(half, la)()
                        elif half == 0 and la - NG < W_LOOKAHEAD:
                            w_load(1, la - NG)()
                        w_t = st.pop(("w", half, g))
                        for j in range(2):
                            kt = 2 * g + j
                            first = g == 0 and j == 0
                            last = g == NG - 1 and j == 1
                            for m in range(4):
                                nc.tensor.matmul(
                                    st["ps1"][m][:],
                                    w_t[:, j, P * m:P * (m + 1)],
                                    st["h"][:, kt, :],
                                    start=first, stop=last)
                    return fn

                def rope(m, kind, dst_fn):
                    # kind: "q" | "k" | "v"
                    def fn():
                        ps1 = st["ps1"][m]
                        if kind == "v":
                            vT = work.tile([P, QCW], f16, tag="vT", bufs=2,
                                           name="vT")
                            nc.vector.tensor_scalar(
                                vT[:], ps1[:], CLIP, -CLIP, min_op, max_op)
                            nc.sync.dma_start_transpose(
                                v_sb[:, 4 * qc:4 * (qc + 1), :], vT[:])
                            return
                        a_t = work.tile([P, QCW], f32, tag="ropeA", bufs=2,
                                        name="a_t")
                        nc.vector.tensor_scalar(
                            a_t[:], ps1[:], CLIP, -CLIP, min_op, max_op)
                        b_t = work.tile([P, QCW], f32, tag="ropeB", bufs=2,
                                        name="b_t")
                        nc.gpsimd.dma_start(b_t[0:64, :], a_t[64:128, :])
                        nc.gpsimd.dma_start(b_t[64:128, :], a_t[0:64, :])
                        tabs = st["tabs"]
                        cc_t = tabs["cck"] if kind == "k" else tabs["ccq"]
                        ss_t = tabs["ssk"] if kind == "k" else tabs["ssq"]
                        e_t = work.tile([P, QCW], f32, tag="ropeE", bufs=2,
                                        name="e_t")
                        nc.vector.tensor_tensor(e_t[:], a_t[:], cc_t[:],
                                                mult_op)
                        f_t = work.tile([P, QCW], f32, tag="ropeF", bufs=2,
                                        name="f_t")
                        nc.vector.tensor_tensor(f_t[:], b_t[:], ss_t[:],
                                                mult_op)
                        nc.vector.tensor_tensor(dst_fn(), e_t[:], f_t[:],
                                                add_op)
                    return fn

                def h_load():
                    h = work.tile([P, DT, QCW], f16, tag="h_all", bufs=1,
                                  name="h_all")
                    st["h"] = h
                    for gg in range(8):
                        nc.gpsimd.dma_start(
                            h[:, 6 * gg:6 * (gg + 1), :],
                            hiddenR[:, 6 * gg:6 * (gg + 1), cs])
                st["h_load"] = h_load

                def prefetch():
                    for g in range(W_LOOKAHEAD):
                        w_load(0, g)()
                st["prefetch"] = prefetch

                KT_NS = 8 * 512 * MM_NS
                # pass A: k, v, q0, q1
                for g in range(NG):
                    items.append((KT_NS, kt_group(0, g)))
                items.append((0, rope(0, "k", lambda: k_sb[:, cs])))
                items.append((0, rope(1, "v", None)))
                for m, h6 in ((2, 0), (3, 1)):
                    items.append((0, rope(
                        m, "q", lambda h6=h6: st["q"][:, h6, :])))
                # pass B: q2..q5
                for g in range(NG):
                    items.append((KT_NS, kt_group(1, g)))
                for m, h6 in ((0, 2), (1, 3), (2, 4), (3, 5)):
                    items.append((0, rope(
                        m, "q", lambda h6=h6: st["q"][:, h6, :])))
                st["items"] = items
                return st

            # ---------------- stage 2: attention ----------------
            def make_s2(qc, s1_state):
                items = []
                st = {"attnT": None}

                def start():
                    st["attnT"] = work.tile([P, NQH, QCW], f16, tag="attnT",
                                            bufs=2, name="attnT")
                items.append((0, start))

                njt = 4 * (qc + 1)
                for h in range(NQH):
                    def head_start(h=h):
                        pT = work.tile([P, NKT, QCW], f16, tag="probsT",
                                       bufs=2, name="probsT")
                        for jl in range(1, 4):
                            nc.vector.memset(
                                pT[:, 4 * qc + jl, :P * jl], 0.0)
                        st[("pT", h)] = pT
                    items.append((0, head_start))

                    for il in range(4):
                        i = 4 * qc + il
                        L = P * (i + 1)
                        nkc = (L + 511) // 512

                        def scores(h=h, il=il, i=i, L=L, nkc=nkc):
                            pT = st[("pT", h)]
                            q_qc = s1_state["q"]
                            s_all = stats.tile([P, 4], f32, tag="s_all",
                                               bufs=4, name="s_all")
                            probs16 = work.tile([P, S], f16, tag="probs16",
                                                bufs=2, name="probs16")
                            pscs = []
                            for kc in range(nkc):
                                n = min(512, L - 512 * kc)
                                last = kc == nkc - 1
                                psc = psp.tile([P, 512], f32, tag="sc",
                                               bufs=2, name="psc")
                                nc.tensor.matmul(
                                    psc[:, :n],
                                    q_qc[:, h, P * il:P * (il + 1)],
                                    k_sb[:, 512 * kc:512 * kc + n],
                                    start=True, stop=not last)
                                if last:
                                    nc.tensor.matmul(
                                        psc[:, n - P:n], ident_sb[:],
                                        maskd_sb[:], start=False, stop=True)
                                pscs.append((psc, n, kc))
                            p32s = []
                            for psc, n, kc in pscs:
                                p32 = work.tile([P, 512], f32, tag="p32",
                                                bufs=4, name="p32")
                                nc.scalar.activation(
                                    p32[:, :n], psc[:, :n], Exp,
                                    bias=negcap[:], scale=1.0,
                                    accum_out=s_all[:, kc:kc + 1])
                                p32s.append((p32, n, kc))
                            ssum = stats.tile([P, 1], f32, tag="ssum",
                                              bufs=4, name="ssum")
                            nc.vector.reduce_sum(ssum[:], s_all[:, :nkc],
                                                 axis=X)
                            rcp = stats.tile([P, 1], f32, tag="rcp",
                                             bufs=4, name="rcp")
                            nc.vector.reciprocal(rcp[:], ssum[:])
                            for p32, n, kc in p32s:
                                nc.vector.tensor_scalar_mul(
                                    probs16[:, 512 * kc:512 * kc + n],
                                    p32[:, :n], rcp[:])
                            nc.sync.dma_start_transpose(
                                pT[:, :i + 1, P * il:P * (il + 1)],
                                probs16[:, :L])
                        items.append((L * MM_NS + 190, scores))

                    def pv(h=h):
                        pT = st.pop(("pT", h))
                        ps_pv = psp.tile([P, 512], f32, tag="pvo", bufs=2,
                                         name="ps_pv")
                        for j in range(njt):
                            nc.tensor.matmul(
                                ps_pv[:], v_sb[:, j, :], pT[:, j, :],
                                start=(j == 0), stop=(j == njt - 1))
                        nc.vector.tensor_copy(st["attnT"][:, h, :],
                                              ps_pv[:])
                    items.append((njt * 512 * MM_NS, pv))

                return {"items": items, "st": st}

            # ---------------- stage 3: outproj + RS ----------------
            WO_LOOKAHEAD = 3

            def make_s3(qc, s2_state):
                items = []
                st = {}

                def start():
                    st["outT"] = dram.tile([D, QCW], f16, tag="outT",
                                           bufs=2, name="outT")
                    st["rs"] = dram.tile([D // N_CORES, QCW], f16,
                                         tag="rsout", bufs=2, name="rsout")
                    for g in range(WO_LOOKAHEAD):
                        wo_load(g)()
                items.append((0, start))

                def wo_load(g):
                    def fn():
                        t = stream.tile([P, 2, NQH, P], f16, tag="wo",
                                        bufs=6, name="wo_t")
                        nc.gpsimd.dma_start(t[:], woutR[:, g, :])
                        st[("wo", g)] = t
                    return fn

                def dm_unit(dm):
                    def fn():
                        g, j = dm // 2, dm % 2
                        if j == 0:
                            la = g + WO_LOOKAHEAD
                            if la < NG:
                                wo_load(la)()
                        wo_t = st[("wo", g)]
                        if j == 1:
                            st.pop(("wo", g))
                        pso = psp.tile([P, 512], f32, tag="pvo", bufs=2,
                                       name="pso")
                        attnT = s2_state["st"]["attnT"]
                        for h6 in range(NQH):
                            nc.tensor.matmul(
                                pso[:], wo_t[:, j, h6, :], attnT[:, h6, :],
                                start=(h6 == 0), stop=(h6 == NQH - 1))
                        ot = work.tile([P, QCW], f16, tag="ot", bufs=4,
                                       name="ot")
                        if dm % 2 == 0:
                            nc.scalar.copy(ot[:], pso[:])
                        else:
                            nc.vector.tensor_copy(ot[:], pso[:])
                        nc.gpsimd.dma_start(
                            st["outT"][P * dm:P * (dm + 1), :], ot[:])
                    return fn

                def rs_half(hf):
                    def fn():
                        rows = slice(3072 * hf, 3072 * (hf + 1))
                        orows = slice(384 * hf, 384 * (hf + 1))
                        nc.gpsimd.collective_compute(
                            "ReduceScatter",
                            mybir.AluOpType.add,
                            replica_groups=[list(range(N_CORES))],
                            ins=[st["outT"][rows, :]],
                            outs=[st["rs"][orows, :]],
                        )
                        nc.sync.dma_start(outs[qc][orows, :],
                                          st["rs"][orows, :])
                    return fn

                DM_NS = 6 * 512 * MM_NS
                for dm in range(DT):
                    items.append((DM_NS, dm_unit(dm)))
                    if dm == 23:
                        items.append((0, rs_half(0)))
                items.append((0, rs_half(1)))
                return {"items": items}

            # ---------------- run the pipeline ----------------
            s1 = [None] * NQC
            s2 = [None] * NQC

            s1[0] = make_s1(0)
            s1[0]["prefetch"]()
            s1[0]["h_load"]()
            for p in range(NQC + 2):
                lanes = []
                if p < NQC:
                    if s1[p] is None:
                        s1[p] = make_s1(p)
                        s1[p]["prefetch"]()
                        s1[p]["h_load"]()
                    lanes.append(s1[p]["items"])
                if 1 <= p <= NQC:
                    s2[p - 1] = make_s2(p - 1, s1[p - 1])
                    lanes.append(s2[p - 1]["items"])
                if p >= 2:
                    lanes.append(make_s3(p - 2, s2[p - 2])["items"])
                merge(*lanes)
                # next chunk's hidden + weights begin loading once this
                # chunk's pass B has consumed the current buffers
                if p + 1 < NQC:
                    s1[p + 1] = make_s1(p + 1)
                    s1[p + 1]["h_load"]()
                    s1[p + 1]["prefetch"]()

    nc.compile()
    return nc


def _get_nc():
    global _cached_nc
    if _cached_nc is None:
        _cached_nc = _build_nc()
    return _cached_nc


def kernel(**inputs):
    from concourse.bass_utils import run_bass_kernel_spmd

    hs = np.asarray(inputs["hidden_states"])[0].astype(np.float32)   # [S, D]
    Wqkv = np.asarray(inputs["Wqkv"]).astype(np.float32)             # [8192, D]
    Wout = np.asarray(inputs["Wout"]).astype(np.float32)             # [D, D]
    pos = np.asarray(inputs["position_ids"])[0]

    f16 = np.float16
    WT = Wqkv.T.astype(f16)                                          # [D, 8192]
    WoT = Wout.T.astype(f16)                                         # [D, D]

    # hiddenR: [128, 48, 2048] -- partition-major repack of hidden^T
    hT = np.ascontiguousarray(hs.T).astype(f16)                      # [D, S]
    hiddenR = np.ascontiguousarray(
        hT.reshape(DT, P, S).transpose(1, 0, 2))

    half = HD // 2
    inv = (1.0 / (500000.0 ** (np.arange(half, dtype=np.float32) * 2.0 / HD)))
    ang = pos.astype(np.float32)[:, None] * inv[None, :].astype(np.float32)
    cos = np.cos(ang).T.astype(np.float32)                           # [64, S]
    sin = np.sin(ang).T.astype(np.float32)
    cc = np.concatenate([cos, cos], axis=0)                          # [128, S]
    ss = np.concatenate([-sin, sin], axis=0)
    ccq = np.ascontiguousarray((cc * SCALE).astype(f16))
    ssq = np.ascontiguousarray((ss * SCALE).astype(f16))
    cck = np.ascontiguousarray(cc.astype(f16))
    ssk = np.ascontiguousarray(ss.astype(f16))
    idx = np.arange(P)
    identm = np.eye(P, dtype=np.float16)
    maskdm = np.where(idx[None, :] > idx[:, None], -60000.0,
                      0.0).astype(np.float16)

    in_maps = []
    for c in range(N_CORES):
        # per-core Wqkv columns reordered to [k(128), v(128), q0..q5(768)]
        wq = np.concatenate([
            WT[:, D + P * c:D + P * (c + 1)],                # k head c
            WT[:, D + 1024 + P * c:D + 1024 + P * (c + 1)],  # v head c
            WT[:, 768 * c:768 * (c + 1)],                    # q heads
        ], axis=1)                                           # [D, 1024]
        wqkvR = np.ascontiguousarray(
            wq.reshape(NG, 2, P, 1024).transpose(2, 0, 1, 3))
        wo = WoT[768 * c:768 * (c + 1), :]                   # [768, D]
        woutR = np.ascontiguousarray(
            wo.reshape(NQH, P, DT, P).transpose(1, 2, 0, 3)  # [p, dm, h, c]
            .reshape(P, NG, 2 * NQH * P))
        in_maps.append(dict(hiddenR=hiddenR, wqkvR=wqkvR, woutR=woutR,
                            ccq=ccq, ssq=ssq, cck=cck, ssk=ssk,
                            ident=identm, maskd=maskdm))

    nc = _get_nc()
    res = run_bass_kernel_spmd(nc, in_maps, core_ids=list(range(N_CORES)))
    kernel._last_results = res

    # out{qc}[384*hf + r] holds global rows 3072*hf + 384*c + r
    outT = np.empty((D, S), np.float32)
    for qc in range(NQC):
        for c in range(N_CORES):
            o = res.results[c][f"out{qc}"].astype(np.float32)
            for hf in range(2):
                outT[3072 * hf + 384 * c:3072 * hf + 384 * (c + 1),
                     QCW * qc:QCW * (qc + 1)] = o[384 * hf:384 * (hf + 1)]
    return np.ascontiguousarray(outT.T)[None]
